# revision 1
# baseline (speedup 1.0000x reference)
"""APPNP-over-GAT distributed Trainium2 kernel (8 NeuronCores).

Sharding: tensor-parallel over (head, out_feat). Each core owns a 128-wide
slice of every head's 1024 out-features (3*128 = 384 local features).
- GAT projection h = x_norm @ W computed locally on the feature shard in
  bf16 (host casts x/W/A to bf16; f32 PSUM accumulation).
- x and W are host-packed into one [16, 128, 3584] stream so the whole
  projection needs 16 dma_starts (descriptor issue was the v1 bottleneck).
- el/er attention partial dots -> AllReduce [512,6] f32.
- Attention in transposed [src j, dst i] layout: softmax denominators via
  ones-vector matmuls; no max-subtract (logits are tiny, exp can't
  overflow; masked entries are exactly zeroed by the A^T multiply).
- k=10 APPNP steps on the local [512, 384] shard, bf16 matmuls with f32
  teleport term; last step emits f32.
- fc: local partial dot against the matching fc_w slice -> AllReduce.
"""

import os
import sys

sys.path.insert(0, "/opt/trn_rl_repo")

import numpy as np

N = 500
NP = 512  # padded nodes
F = 8192
H = 3
O = 1024
OL = 128  # out-features per head per core
SH = H * OL  # 384 local features
KF = F // 128  # 64 k-tiles
G = 4  # k-tiles per DMA group
NG = KF // G  # 16 groups
XWC = NP + SH  # 896 columns per k-tile in the fused x|w stream
NC = 8
K_STEPS = 10
ALPHA = 0.1
NEG_SLOPE = 0.2

LAST_EXEC_NS = None
LAST_RESULT = None


def build():
    import concourse.bacc as bacc
    import concourse.mybir as mybir
    import concourse.tile as tile
    from concourse.masks import make_identity

    f32 = mybir.dt.float32
    bf16 = mybir.dt.bfloat16
    Alu = mybir.AluOpType
    Act = mybir.ActivationFunctionType
    AX = mybir.AxisListType.X

    nc = bacc.Bacc("TRN2", target_bir_lowering=False, debug=False, num_devices=NC)

    xw = nc.declare_dram_parameter("xw", [NG, 128, G * XWC], bf16, isOutput=False)
    aftp = nc.declare_dram_parameter("aftp", [128, 4 * NP], bf16, isOutput=False)
    attn = nc.declare_dram_parameter("attn", [1, 2 * SH], f32, isOutput=False)
    fcwp = nc.declare_dram_parameter("fcwp", [128, 4 * 2 * SH], f32, isOutput=False)
    fcb = nc.declare_dram_parameter("fcb", [1, 16], f32, isOutput=False)
    out_ext = nc.declare_dram_parameter("out", [1, 16], f32, isOutput=True)

    rg = [list(range(NC))]

    with tile.TileContext(nc) as tc:
        with (
            tc.tile_pool(name="consts", bufs=1) as consts,
            tc.tile_pool(name="persist", bufs=1) as persist,
            tc.tile_pool(name="stream", bufs=3) as stream,
            tc.tile_pool(name="dram", bufs=1, space="DRAM") as dram,
        ):
            ident = consts.tile([128, 128], bf16, name="ident", tag="ident")
            make_identity(nc, ident[:, :])
            ones_col = consts.tile([128, 1], f32, name="ones_col", tag="ones_col")
            nc.gpsimd.memset(ones_col[:, :], 1.0)
            ones_col_b = consts.tile([128, 1], bf16, name="ones_col_b", tag="ocb")
            nc.gpsimd.memset(ones_col_b[:, :], 1.0)
            ones_row = consts.tile([1, 128], f32, name="ones_row", tag="ones_row")
            nc.gpsimd.memset(ones_row[:, :], 1.0)

            # ---- early DMAs (gpsimd queue, off the sync stream queue)
            aft_sb = persist.tile([128, 4 * NP], bf16, name="aft_sb", tag="aft_sb")
            nc.gpsimd.dma_start(aft_sb[:, :], aftp[:, :])
            aft_t = [aft_sb[:, k * NP : (k + 1) * NP] for k in range(4)]
            attn_sb = consts.tile([1, 2 * SH], f32, name="attn_sb", tag="attn_sb")
            nc.gpsimd.dma_start(attn_sb[:, :], attn[:, :])
            fcb_sb = consts.tile([1, 16], f32, name="fcb_sb", tag="fcb_sb")
            nc.gpsimd.dma_start(fcb_sb[:, :], fcb[:, :])
            fcw_sb = persist.tile([128, 8 * SH], f32, name="fcw_sb", tag="fcw_sb")
            nc.gpsimd.dma_start(fcw_sb[:, :], fcwp[:, :])
            fcw_t = [fcw_sb[:, m * 2 * SH : (m + 1) * 2 * SH] for m in range(4)]

            # ---- degree (colsums of A^T) -> dinv row + broadcast + cols
            ppA = tc.tile_pool(name="psumA", bufs=1, space="PSUM")
            pp = ppA.__enter__()
            d_psum = pp.tile([1, NP], f32, name="d_psum", tag="d_psum")
            for k in range(4):
                nc.tensor.matmul(
                    d_psum[:, :], ones_col_b[:, :], aft_t[k],
                    start=(k == 0), stop=(k == 3),
                )
            dinv_row = consts.tile([1, NP], f32, name="dinv_row", tag="dinv_row")
            nc.scalar.activation(dinv_row[:, :], d_psum[:, :], Act.Sqrt, bias=1.0)
            nc.vector.reciprocal(dinv_row[:, :], dinv_row[:, :])
            dinvb_psum = pp.tile([128, NP], f32, name="dinvb_psum", tag="dinvb_psum")
            nc.tensor.matmul(dinvb_psum[:, :], ones_row[:, :], dinv_row[:, :])
            dinvb = persist.tile([128, NP], f32, name="dinvb", tag="dinvb")
            nc.vector.tensor_copy(dinvb[:, :], dinvb_psum[:, :])
            dinv_dram = dram.tile([1, NP], f32, name="dinv_dram", tag="dinv_dram")
            nc.sync.dma_start(dinv_dram[:, :], dinv_row[:, :])
            dinvc = persist.tile([128, 4], f32, name="dinvc", tag="dinvc")
            nc.sync.dma_start(
                dinvc[:, :], dinv_dram.rearrange("a (k p) -> p (a k)", p=128)
            )

            # ---- fused projection stream: h = (x / l1colsum(x)) @ W
            hp_psum = [pp.tile([128, SH], f32, name=f"hp{m}", tag=f"hp{m}") for m in range(4)]
            for g in range(NG):
                xwt = stream.tile([128, G * XWC], bf16, name="xwt", tag="xwt")
                nc.sync.dma_start(xwt[:, :], xw[g, :, :])
                for j in range(G):
                    k = g * G + j
                    xk = xwt[:, j * XWC : j * XWC + NP]
                    wk = xwt[:, j * XWC + NP : (j + 1) * XWC]
                    # |x| on ScalarE with fused free-axis accumulate -> s_k
                    s_k = stream.tile([128, 1], f32, name="sk", tag="sk", bufs=8)
                    absj = stream.tile([128, NP], bf16, name="absj", tag="absj")
                    nc.scalar.activation(
                        absj[:, :], xk, Act.Abs, accum_out=s_k[:, :]
                    )
                    rs_k = stream.tile([128, 1], f32, name="rsk", tag="rsk", bufs=8)
                    nc.vector.reciprocal(rs_k[:, :], s_k[:, :])
                    wks = stream.tile([128, SH], bf16, name="wks", tag="wks", bufs=6)
                    nc.any.tensor_scalar_mul(wks[:, :], wk, rs_k[:, :])
                    for m in range(4):
                        nc.tensor.matmul(
                            hp_psum[m][:, :],
                            xwt[:, j * XWC + m * 128 : j * XWC + (m + 1) * 128],
                            wks[:, :],
                            start=(k == 0),
                            stop=(k == KF - 1),
                        )
            hp_sb = [persist.tile([128, SH], bf16, name=f"hpsb{m}", tag=f"hpsb{m}") for m in range(4)]
            for m in range(4):
                nc.any.tensor_copy(hp_sb[m][:, :], hp_psum[m][:, :])

            # ---- el/er partial dots + AllReduce
            attnl_psum = pp.tile([128, SH], f32, name="attnl_psum", tag="alp")
            nc.tensor.matmul(attnl_psum[:, :], ones_row[:, :], attn_sb[:, 0:SH])
            attnr_psum = pp.tile([128, SH], f32, name="attnr_psum", tag="arp")
            nc.tensor.matmul(attnr_psum[:, :], ones_row[:, :], attn_sb[:, SH : 2 * SH])
            attnl_sb = persist.tile([128, SH], bf16, name="attnl_sb", tag="attnl_sb")
            nc.any.tensor_copy(attnl_sb[:, :], attnl_psum[:, :])
            attnr_sb = persist.tile([128, SH], bf16, name="attnr_sb", tag="attnr_sb")
            nc.any.tensor_copy(attnr_sb[:, :], attnr_psum[:, :])
            eler_in = dram.tile([NP, 6], f32, name="eler_in", tag="eler_in")
            eler_out = dram.tile([NP, 6], f32, name="eler_out", tag="eler_out")
            for m in range(4):
                prod = stream.tile([128, SH], bf16, name="elprod", tag="elprod")
                eler_m = stream.tile([128, 6], f32, name="eler_m", tag="eler_m")
                nc.vector.tensor_mul(prod[:, :], hp_sb[m][:, :], attnl_sb[:, :])
                nc.vector.reduce_sum(
                    eler_m[:, 0:3], prod.rearrange("p (h o) -> p h o", h=H), axis=AX
                )
                nc.vector.tensor_mul(prod[:, :], hp_sb[m][:, :], attnr_sb[:, :])
                nc.vector.reduce_sum(
                    eler_m[:, 3:6], prod.rearrange("p (h o) -> p h o", h=H), axis=AX
                )
                nc.sync.dma_start(eler_in[m * 128 : (m + 1) * 128, :], eler_m[:, :])
            ppA.__exit__(None, None, None)
            nc.gpsimd.collective_compute(
                "AllReduce", Alu.add, ins=[eler_in.opt()], outs=[eler_out.opt()],
                replica_groups=rg,
            )
            ppB = tc.tile_pool(name="psumB", bufs=1, space="PSUM")
            pp = ppB.__enter__()
            # readback: er as per-partition cols [128, (k,6)], el as rows [3, 512]
            erl = persist.tile([128, 4, 6], f32, name="erl", tag="erl")
            nc.sync.dma_start(erl[:, :, :], eler_out.rearrange("(k p) c -> p k c", p=128))
            el_rows = [
                persist.tile([1, NP], f32, name=f"el_row{h}", tag=f"el_row{h}")
                for h in range(H)
            ]
            for h in range(H):
                nc.sync.dma_start(
                    el_rows[h][:, :], eler_out[:, h : h + 1].rearrange("n h -> h n")
                )

            # ---- attention numerators, [j, i] layout, bf16
            num_t = {}
            for h in range(H):
                elb_psum = pp.tile([128, NP], f32, name="elb", tag="elb", bufs=2)
                nc.tensor.matmul(elb_psum[:, :], ones_row[:, :], el_rows[h][:, :])
                for k in range(4):
                    e_t = stream.tile([128, NP], f32, name="esc", tag="esc")
                    numb = persist.tile([128, NP], bf16, name=f"num{h}_{k}", tag=f"num{h}_{k}")
                    # z = el_i + er_j ; lrelu(z) = max(0.2*z, z) ; num = A^T * exp
                    nc.scalar.activation(
                        e_t[:, :], elb_psum[:, :], Act.Identity,
                        bias=erl[:, k, 3 + h : 4 + h],
                    )
                    nc.vector.scalar_tensor_tensor(
                        e_t[:, :], e_t[:, :], NEG_SLOPE, e_t[:, :],
                        op0=Alu.mult, op1=Alu.max,
                    )
                    nc.scalar.activation(e_t[:, :], e_t[:, :], Act.Exp)
                    nc.vector.tensor_mul(numb[:, :], e_t[:, :], aft_t[k])
                    num_t[(h, k)] = numb

            # ---- softmax denominators (column sums) -> 1/den cols
            rd_dram = dram.tile([H, NP], f32, name="rd_dram", tag="rd_dram")
            for h in range(H):
                den_psum = pp.tile([1, NP], f32, name="den", tag="den", bufs=2)
                for k in range(4):
                    nc.tensor.matmul(
                        den_psum[:, :], ones_col_b[:, :], num_t[(h, k)][:, :],
                        start=(k == 0), stop=(k == 3),
                    )
                rd_row = stream.tile([1, NP], f32, name="rdrow", tag="rdrow")
                # +1e-30 keeps padded (all-masked) columns finite: 1/1e-30=1e30,
                # and padded h0 rows are exactly 0, so 0 * 1e30 = 0.
                nc.vector.tensor_scalar_add(rd_row[:, :], den_psum[:, :], 1e-30)
                nc.vector.reciprocal(rd_row[:, :], rd_row[:, :])
                nc.sync.dma_start(rd_dram[h : h + 1, :], rd_row[:, :])
            rdc = persist.tile([128, 4, H], f32, name="rdc", tag="rdc")
            for h in range(H):
                nc.sync.dma_start(
                    rdc[:, :, h],
                    rd_dram[h : h + 1, :].rearrange("a (m p) -> p (a m)", p=128),
                )
            rd01 = persist.tile([128, 4, H], f32, name="rd01", tag="rd01")
            nc.vector.tensor_scalar_mul(rd01[:, :, :], rdc[:, :, :], ALPHA)

            # ---- h0 = att @ h (rows scaled by 1/den); bf16 copy + f32 alpha term
            h0_psum = [pp.tile([128, SH], f32, name=f"h0p{m}", tag=f"h0p{m}") for m in range(4)]
            for m in range(4):
                for h in range(H):
                    for k in range(4):
                        nc.tensor.matmul(
                            h0_psum[m][:, h * OL : (h + 1) * OL],
                            num_t[(h, k)][:, m * 128 : (m + 1) * 128],
                            hp_sb[k][:, h * OL : (h + 1) * OL],
                            start=(k == 0),
                            stop=(k == 3),
                        )
            h0_sb = [persist.tile([128, SH], bf16, name=f"h0sb{m}", tag=f"h0sb{m}") for m in range(4)]
            h0s = [persist.tile([128, SH], f32, name=f"h0s{m}", tag=f"h0s{m}") for m in range(4)]
            for m in range(4):
                for h in range(H):
                    nc.scalar.mul(
                        h0_sb[m][:, h * OL : (h + 1) * OL],
                        h0_psum[m][:, h * OL : (h + 1) * OL],
                        rdc[:, m, h : h + 1],
                    )
                    nc.scalar.mul(
                        h0s[m][:, h * OL : (h + 1) * OL],
                        h0_psum[m][:, h * OL : (h + 1) * OL],
                        rd01[:, m, h : h + 1],
                    )
            ppB.__exit__(None, None, None)

            # ---- Ahat^T = (A^T + I) * dinv_j * dinv_i  (in place, bf16)
            for k in range(4):
                nc.vector.tensor_add(
                    aft_sb[:, k * NP + k * 128 : k * NP + (k + 1) * 128],
                    aft_sb[:, k * NP + k * 128 : k * NP + (k + 1) * 128],
                    ident[:, :],
                )
                nc.vector.scalar_tensor_tensor(
                    aft_t[k], aft_t[k], dinvc[:, k : k + 1], dinvb[:, :],
                    op0=Alu.mult, op1=Alu.mult,
                )

            # ---- APPNP: 10x  hc = 0.9 * Ahat @ hc + 0.1 * h0
            ppC = tc.tile_pool(name="psumC", bufs=1, space="PSUM")
            pp = ppC.__enter__()
            hc = h0_sb
            for t in range(K_STEPS):
                last = t == K_STEPS - 1
                nxt = []
                for m in range(4):
                    ap_psum = pp.tile([128, SH], f32, name=f"ap{m}", tag=f"ap{m}")
                    for k in range(4):
                        nc.tensor.matmul(
                            ap_psum[:, :],
                            aft_sb[:, k * NP + m * 128 : k * NP + (m + 1) * 128],
                            hc[k][:, :],
                            start=(k == 0),
                            stop=(k == 3),
                        )
                    hk_m = persist.tile(
                        [128, SH], f32 if last else bf16,
                        name=f"hk{m}", tag=f"hk{m}_{t % 2}_{last}",
                    )
                    nc.vector.scalar_tensor_tensor(
                        hk_m[:, :], ap_psum[:, :], 1.0 - ALPHA, h0s[m][:, :],
                        op0=Alu.mult, op1=Alu.add,
                    )
                    nxt.append(hk_m)
                hc = nxt

            # ---- fc: partial dot + AllReduce + bias
            parts = stream.tile([128, 8], f32, name="parts", tag="parts")
            for m in range(4):
                for c in range(2):
                    junk = stream.tile([128, SH], f32, name="fcjunk", tag="fcjunk")
                    nc.vector.tensor_mul(
                        junk[:, :], hc[m][:, :], fcw_t[m][:, c * SH : (c + 1) * SH]
                    )
                    nc.vector.reduce_sum(
                        parts[:, c * 4 + m : c * 4 + m + 1], junk[:, :], axis=AX
                    )
            fin_psum = pp.tile([1, 8], f32, name="fin", tag="fin")
            nc.tensor.matmul(fin_psum[:, :], ones_col[:, :], parts[:, :])
            res256 = stream.tile([1, 256], f32, name="res256", tag="res256")
            nc.gpsimd.memset(res256[:, :], 0.0)
            nc.vector.reduce_sum(
                res256[:, 0:2], fin_psum.rearrange("p (b c) -> p b c", b=2), axis=AX
            )
            ppC.__exit__(None, None, None)
            fc_in = dram.tile([1, 256], f32, name="fc_in", tag="fc_in")
            fc_out = dram.tile([1, 256], f32, name="fc_out", tag="fc_out")
            nc.sync.dma_start(fc_in[:, :], res256[:, :])
            nc.gpsimd.collective_compute(
                "AllReduce", Alu.add, ins=[fc_in.opt()], outs=[fc_out.opt()],
                replica_groups=rg,
            )
            res_f = stream.tile([1, 16], f32, name="resf", tag="resf")
            nc.sync.dma_start(res_f[:, :], fc_out[0:1, 0:16])
            nc.vector.tensor_add(res_f[:, :], res_f[:, :], fcb_sb[:, :])
            nc.sync.dma_start(out_ext[:, :], res_f[:, :])

    nc.finalize()
    return nc


def prepare_in_maps(A, x, W, attn_l, attn_r, fc_w, fc_b):
    import ml_dtypes

    bf16 = ml_dtypes.bfloat16
    A = np.asarray(A)
    x = np.asarray(x, dtype=np.float32)
    W = np.asarray(W, dtype=np.float32)
    attn_l = np.asarray(attn_l, dtype=np.float32)
    attn_r = np.asarray(attn_r, dtype=np.float32)
    fc_w = np.asarray(fc_w, dtype=np.float32)
    fc_b = np.asarray(fc_b, dtype=np.float32)

    xT = np.zeros((F, NP), dtype=bf16)
    xT[:, :N] = x.T.astype(bf16)
    aft = np.zeros((NP, NP), dtype=bf16)
    aft[:N, :N] = A.T.astype(bf16)
    # [128, (k, 512)] packing: partition p, col k*512+j = aft[k*128+p, j]
    aftp = np.ascontiguousarray(
        aft.reshape(4, 128, NP).transpose(1, 0, 2).reshape(128, 4 * NP)
    )
    fcb = np.zeros((1, 16), dtype=np.float32)
    fcb[0, :2] = fc_b
    fcv = fc_w.reshape(2, N, H, O)

    in_maps = []
    for c in range(NC):
        sl = slice(c * OL, (c + 1) * OL)
        w_c = W[:, :, sl].transpose(1, 0, 2).reshape(F, SH).astype(bf16)
        # fused x|w stream: [NG, 128, (j, 896)]
        xwf = np.concatenate([xT, w_c], axis=1)  # [F, 896]
        xwg = np.ascontiguousarray(
            xwf.reshape(NG, G, 128, XWC).transpose(0, 2, 1, 3).reshape(NG, 128, G * XWC)
        )
        attn_c = np.concatenate(
            [attn_l[:, sl].reshape(-1), attn_r[:, sl].reshape(-1)]
        ).reshape(1, 2 * SH).astype(np.float32)
        fcw_c = np.zeros((NP, 2 * SH), dtype=np.float32)
        fcw_c[:N, :] = fcv[:, :, :, sl].transpose(1, 0, 2, 3).reshape(N, 2 * SH)
        fcwp = np.ascontiguousarray(
            fcw_c.reshape(4, 128, 2 * SH).transpose(1, 0, 2).reshape(128, 8 * SH)
        )
        in_maps.append(
            {"xw": xwg, "aftp": aftp, "attn": attn_c, "fcwp": fcwp, "fcb": fcb}
        )
    return in_maps


def _ensure_ntff_hook():
    """The agent image's antenv lacks axon_hooks; register the profile hook
    ourselves so run_bass_kernel_spmd(trace=True) can collect NTFF profiles."""
    import types

    try:
        from antenv.axon_hooks import get_axon_ntff_profile_hook  # noqa: F401
        return
    except ImportError:
        pass
    try:
        import antenv
        from trn_agent_boot.trn_boot import _ntff_profile_via_ctypes

        mod = types.ModuleType("antenv.axon_hooks")
        _hook = [_ntff_profile_via_ctypes("/opt/axon/libaxon_pjrt.so")]
        mod.set_axon_ntff_profile_hook = lambda h: _hook.__setitem__(0, h)
        mod.get_axon_ntff_profile_hook = lambda: _hook[0]
        sys.modules["antenv.axon_hooks"] = mod
        antenv.axon_hooks = mod
    except Exception:
        pass


def kernel(A, x, W, attn_l, attn_r, fc_w, fc_b):
    global LAST_EXEC_NS, LAST_RESULT
    from concourse.bass_utils import run_bass_kernel_spmd

    if os.environ.get("BASS_TRACE"):
        _ensure_ntff_hook()

    in_maps = prepare_in_maps(A, x, W, attn_l, attn_r, fc_w, fc_b)
    nc = build()
    res = run_bass_kernel_spmd(
        nc, in_maps, core_ids=list(range(NC)),
        trace=bool(os.environ.get("BASS_TRACE")),
    )
    LAST_EXEC_NS = res.exec_time_ns
    LAST_RESULT = res
    out = res.results[0]["out"]
    return np.asarray(out).reshape(-1)[:2].astype(np.float32)

